# revision 22
# baseline (speedup 1.0000x reference)
"""Trainium2 Bass kernel for the LUT-linear (embedding_lookup) problem — V4.

Math: per_table[b,t] = (A + B*x0 + C*x1 + D*x0*x1)/4 with
x0 = input[b, mask[2t]], x1 = input[b, mask[2t+1]];
A=w0+w1+w2+w3, B=-w0+w1-w2+w3, C=-w0-w1+w2+w3, D=w0-w1-w2+w3.
out[b,o] = bias[o] + sum_{t in seg_o} per_table (segments = 512 contiguous
tables per out-feature).

Strategy (8 cores, table-sharded; input replicated):
  - Linear + const terms fold on the host into weight-only arrays:
    W_lin[i,o] = sum_t (B/4)[m0=i] + (C/4)[m1=i], const[o] = bias + sum A/4;
    applied with 4 PE matmuls. Only the quadratic term runs per-table.
  - Tables are placed into m0-runs: run i (= rg*128 + rr) owns RCAP=64
    slots at partition rr, w = rg*64+rep. x0 for every slot of run i is
    input[:, i] -> a stride-0 broadcast VIEW of input_sb[rr, rg, :]: no
    gather for x0. Run overflow (~1.7K tables/core) goes to an OV region
    where both x0 and x1 are SWDGE-gathered.
  - x1 is SWDGE-gathered per slot (32 main + 2 OV calls of 1024 idxs).
    Q7 descriptor-gen ucode is the critical path.
  - DVE: one pass per chunk, u = x0_view * x1 (f32 in, bf16 out).
  - Reduce: per-w stationary S_w[p,o] = D/4 routed to the slot's segment.
    Matmuls alternate PE column halves (even w -> PSUM rows 0:64, odd w
    -> rows 64:128) so each LDWEIGHTS overlaps the other half's MATMUL.
  - Epilogue: out = ps_even + ps_odd + const in one DVE op.
"""

import numpy as np
import ml_dtypes

NCORES = 8
B = 64
IN = 512
OUT = 512
T = IN * OUT
TC = T // NCORES          # 32768 tables per core
SEG = 512                 # tables per out-feature
OC = OUT // NCORES        # 64 out-features per core
NPART = 128
RG = IN // NPART          # 4 row-groups
RCAP = 64                 # slots per m0-run
WMAIN = RG * RCAP         # 256 main w-columns
NCHUNK = 8
WCH = WMAIN // NCHUNK     # 32 w-cols per chunk
GIDX = 1024               # idxs per dma_gather call
GW = GIDX // 128          # w-cols covered per gather call
CPC = (WMAIN // NCHUNK) // GW  # gather calls per chunk
NQUEUES = 4
OV_CHUNK_AFTER = 2        # run overflow compute after this chunk

_CACHE = {}


def _build_program(ov_slots, ov_valid):
    import concourse.bacc as bacc
    import concourse.mybir as mybir
    from concourse import library_config
    from concourse.tile import TileContext

    f32 = mybir.dt.float32
    bf16 = mybir.dt.bfloat16
    i16 = mybir.dt.int16
    Alu = mybir.AluOpType

    novc = ov_slots // GIDX
    ovw = ov_slots // NPART

    nc = bacc.Bacc("TRN2", target_bir_lowering=False, debug=False,
                   num_devices=NCORES, num_swdge_queues=NQUEUES,
                   dynamic_dma_scratch_size=65536)

    input_t = nc.dram_tensor("input_t", [IN, B], f32, kind="ExternalInput")
    input_sb_d = nc.dram_tensor("input_sb", [NPART, RG, B], f32, kind="ExternalInput")
    input_bf_d = nc.dram_tensor("input_bf", [NPART, RG, B], bf16, kind="ExternalInput")
    wlin_d = nc.dram_tensor("wlin", [NPART, RG, OC], bf16, kind="ExternalInput")
    const_d = nc.dram_tensor("const", [OC, 1], f32, kind="ExternalInput")
    s_main_d = nc.dram_tensor("s_main", [NCHUNK, NPART, WCH, OC], bf16, kind="ExternalInput")
    s_ov_d = nc.dram_tensor("s_ov", [NPART, ovw, OC], bf16, kind="ExternalInput")
    idx1_d = nc.dram_tensor("idx1", [NCHUNK, NPART, CPC * (GIDX // 16)], i16, kind="ExternalInput")
    idx0ov_d = nc.dram_tensor("idx0ov", [NPART, novc * (GIDX // 16)], i16, kind="ExternalInput")
    idx1ov_d = nc.dram_tensor("idx1ov", [NPART, novc * (GIDX // 16)], i16, kind="ExternalInput")
    out_d = nc.dram_tensor("out_c", [OC, B], f32, kind="ExternalOutput")

    # parity bookkeeping for the two PSUM column-halves
    first_even = [True]
    first_odd = [True]

    with TileContext(nc) as tc:
        nc.gpsimd.load_library(library_config.mlp)
        with (
            tc.tile_pool(name="idx", bufs=1) as idx_pool,
            tc.tile_pool(name="small", bufs=1) as small_pool,
            tc.tile_pool(name="x1", bufs=6) as x1_pool,
            tc.tile_pool(name="u", bufs=4) as u_pool,
            tc.tile_pool(name="s", bufs=3) as s_pool,
            tc.tile_pool(name="ov", bufs=1) as ov_pool,
            tc.tile_pool(name="psum", bufs=1, space="PSUM") as psum_pool,
        ):
            # overflow idx tiles load first so OV gathers can start early;
            # main idx is split per chunk so chunk 0 does not wait on all of it
            idx0ov_sb = idx_pool.tile([NPART, novc * (GIDX // 16)], i16, tag="idx0ov")
            idx1ov_sb = idx_pool.tile([NPART, novc * (GIDX // 16)], i16, tag="idx1ov")
            nc.sync.dma_start(idx0ov_sb[:], idx0ov_d[:])
            nc.sync.dma_start(idx1ov_sb[:], idx1ov_d[:])
            idx1_all = idx_pool.tile([NPART, NCHUNK, CPC * (GIDX // 16)], i16, tag="idx1")
            nc.sync.dma_start(idx1_all[:], idx1_d.rearrange("c p s -> p c s"))
            idx1_sbs = [idx1_all[:, c, :] for c in range(NCHUNK)]

            input_sb = small_pool.tile([NPART, RG, B], f32, tag="input_sb")
            nc.sync.dma_start(input_sb[:], input_sb_d[:])
            wlin_sb = small_pool.tile([NPART, RG, OC], bf16, tag="wlin")
            nc.sync.dma_start(wlin_sb[:], wlin_d[:])
            const_sb = small_pool.tile([OC, 1], f32, tag="const")
            nc.sync.dma_start(const_sb[:], const_d[:])
            s_ov_sb = small_pool.tile([NPART, ovw, OC], bf16, tag="s_ov")
            nc.sync.dma_start(s_ov_sb[:], s_ov_d[:])

            ps = psum_pool.tile([NPART, B], f32, tag="ps")

            def quad_matmul(w_parity, s_ap, u_ap, last=False):
                if w_parity == 0:
                    o_ap = ps[0:OC, :]
                    start = first_even[0]
                    first_even[0] = False
                else:
                    o_ap = ps[OC:2 * OC, :]
                    start = first_odd[0]
                    first_odd[0] = False
                nc.tensor.matmul(o_ap, s_ap, u_ap, start=start, stop=last,
                                 skip_group_check=True)

            # warm-up: a 16-idx gather with a memset idx tile absorbs the
            # SWDGE ucode cold-start while the real idx tiles are still in DMA
            full_reg = nc.gpsimd.to_reg(GIDX)
            # valid counts per OV call (trailing -1 idxs are skipped by ucode)
            ov_counts = [min(GIDX, max(0, ov_valid - k * GIDX)) for k in range(novc)]

            # overflow tiles: memset first — slots beyond the valid count are
            # never written by the gather (-1 idxs) and must not feed NaNs
            # into u*0. The ov gathers themselves are emitted mid-stream
            # (after chunk 1) so chunk 0 compute starts as early as possible.
            x0ov = ov_pool.tile([NPART, ovw, B], f32, tag="x0ov")
            x1ov = ov_pool.tile([NPART, ovw, B], f32, tag="x1ov")
            nc.vector.memset(x0ov[:], 0.0)
            nc.vector.memset(x1ov[:], 0.0)
            qn = 0

            def emit_ov_gathers():
                nonlocal qn
                for k in range(novc):
                    cnt = ov_counts[k]
                    nc.gpsimd.dma_gather(
                        x0ov[:, k * GW:(k + 1) * GW, :], input_t[:],
                        idx0ov_sb[:, k * (GIDX // 16):(k + 1) * (GIDX // 16)],
                        GIDX, full_reg if cnt == GIDX else cnt, B,
                        queue_num=qn % NQUEUES)
                    qn += 1
                    nc.gpsimd.dma_gather(
                        x1ov[:, k * GW:(k + 1) * GW, :], input_t[:],
                        idx1ov_sb[:, k * (GIDX // 16):(k + 1) * (GIDX // 16)],
                        GIDX, full_reg if cnt == GIDX else cnt, B,
                        queue_num=qn % NQUEUES)
                    qn += 1

            emit_ov_gathers()

            # linear part: 4 matmuls on the even half (input_bf is host-cast)
            input_bf = small_pool.tile([NPART, RG, B], bf16, tag="input_bf")
            nc.sync.dma_start(input_bf[:], input_bf_d[:])
            for rg in range(RG):
                quad_matmul(0, wlin_sb[:, rg, :], input_bf[:, rg, :])

            for c in range(NCHUNK):
                s_sb = s_pool.tile([NPART, WCH, OC], bf16, tag="s")
                nc.sync.dma_start(s_sb[:], s_main_d[c])

                rg = c // (NCHUNK // RG)
                if c < NCHUNK - 1:
                    # bulk path: one DVE op per chunk
                    x1t = x1_pool.tile([NPART, WCH, B], f32, tag="x1")
                    for j in range(CPC):
                        nc.gpsimd.dma_gather(
                            x1t[:, j * GW:(j + 1) * GW, :], input_t[:],
                            idx1_sbs[c][:, j * (GIDX // 16):(j + 1) * (GIDX // 16)],
                            GIDX, full_reg, B, queue_num=qn % NQUEUES)
                        qn += 1
                    u = u_pool.tile([NPART, WCH, B], bf16, tag="u")
                    xv = input_sb[:, rg, :].unsqueeze(1).broadcast_to([NPART, WCH, B])
                    nc.vector.tensor_tensor(u[:], xv, x1t[:], Alu.mult)
                    for wl in range(WCH):
                        quad_matmul(wl % 2, s_sb[:, wl, :], u[:, wl, :])
                else:
                    # tail path: consume per gather call to shorten the tail
                    xvg = input_sb[:, rg, :].unsqueeze(1).broadcast_to([NPART, GW, B])
                    for j in range(CPC):
                        x1t = x1_pool.tile([NPART, GW, B], f32, tag="x1s")
                        nc.gpsimd.dma_gather(
                            x1t[:], input_t[:],
                            idx1_sbs[c][:, j * (GIDX // 16):(j + 1) * (GIDX // 16)],
                            GIDX, full_reg, B, queue_num=qn % NQUEUES)
                        qn += 1
                        u = u_pool.tile([NPART, GW, B], bf16, tag="us")
                        nc.vector.tensor_tensor(u[:], xvg, x1t[:], Alu.mult)
                        for wg in range(GW):
                            wl = j * GW + wg
                            last = (j == CPC - 1) and wg >= GW - 2
                            quad_matmul(wl % 2, s_sb[:, wl, :], u[:, wg, :], last=last)

                if c == OV_CHUNK_AFTER:
                    uov = ov_pool.tile([NPART, ovw, B], bf16, tag="uov")
                    nc.vector.tensor_tensor(uov[:], x0ov[:], x1ov[:], Alu.mult)
                    for wl in range(ovw):
                        quad_matmul(wl % 2, s_ov_sb[:, wl, :], uov[:, wl, :])

            out_sb = small_pool.tile([OC, B], f32, tag="out")
            nc.vector.tensor_scalar(out_sb[:], ps[0:OC, :], const_sb[:], None, Alu.add)
            nc.vector.tensor_tensor(out_sb[:], out_sb[:], ps[OC:2 * OC, :], Alu.add)
            nc.sync.dma_start(out_d[:], out_sb[:])

    nc.compile()
    return nc


def _wrap_idx_calls(vals):
    """vals [ncalls, 1024] (position order g*128+p) -> [128, ncalls*64] i16."""
    ncalls = vals.shape[0]
    w = vals.reshape(ncalls, GIDX // 16, 16).transpose(0, 2, 1)
    w = np.tile(w, (1, 8, 1))
    return np.ascontiguousarray(
        w.transpose(1, 0, 2).reshape(NPART, ncalls * (GIDX // 16))
    ).astype(np.int16)


def _prep_core(core, input_mask, lut_weights, bias, ov_slots, ov_valid):
    lo = core * TC
    m0 = input_mask[2 * lo:2 * (lo + TC):2].astype(np.int64)
    m1 = input_mask[2 * lo + 1:2 * (lo + TC):2].astype(np.int64)
    w = lut_weights[lo:lo + TC].astype(np.float32)
    A = (w[:, 0] + w[:, 1] + w[:, 2] + w[:, 3]) * 0.25
    Bc = (-w[:, 0] + w[:, 1] - w[:, 2] + w[:, 3]) * 0.25
    Cc = (-w[:, 0] - w[:, 1] + w[:, 2] + w[:, 3]) * 0.25
    Dc = (w[:, 0] - w[:, 1] - w[:, 2] + w[:, 3]) * 0.25
    seg = np.arange(TC) // SEG

    Wlin = np.zeros((IN, OC), dtype=np.float32)
    np.add.at(Wlin, (m0, seg), Bc)
    np.add.at(Wlin, (m1, seg), Cc)
    const = bias[core * OC:(core + 1) * OC].astype(np.float32).copy()
    np.add.at(const, seg, A)

    d_main = np.zeros((NPART, RG, RCAP), dtype=np.float32)
    m1_main = np.zeros((NPART, RG, RCAP), dtype=np.int64)
    seg_main = np.zeros((NPART, RG, RCAP), dtype=np.int64)
    order = np.argsort(m0, kind="stable")
    counts = np.bincount(m0, minlength=IN)
    starts = np.zeros(IN + 1, dtype=np.int64)
    np.cumsum(counts, out=starts[1:])
    overflow = []
    for i in range(IN):
        tabs = order[starts[i]:starts[i + 1]]
        rr, rg = i % NPART, i // NPART
        fill = min(len(tabs), RCAP)
        tk = tabs[:fill]
        d_main[rr, rg, :fill] = Dc[tk]
        m1_main[rr, rg, :fill] = m1[tk]
        seg_main[rr, rg, :fill] = seg[tk]
        overflow.extend(tabs[RCAP:])
    overflow = np.asarray(overflow, dtype=np.int64)

    assert len(overflow) <= ov_valid <= ov_slots, (len(overflow), ov_valid, ov_slots)
    novc = ov_slots // GIDX
    ovw = ov_slots // NPART
    n = len(overflow)
    # pad with duplicate-dummy valid entries (idx 0, d=0) up to ov_valid so
    # every core shares the same per-call valid count; -1 beyond
    f = np.arange(ov_slots)
    p_of = f % NPART
    w_of = GW * (f // GIDX) + (f % GIDX) // NPART
    d_ovs = np.zeros((NPART, ovw), dtype=np.float32)
    m0_ovs = np.zeros((NPART, ovw), dtype=np.int64)
    m1_ovs = np.zeros((NPART, ovw), dtype=np.int64)
    seg_ovs = np.zeros((NPART, ovw), dtype=np.int64)
    m0_ovs[p_of[ov_valid:], w_of[ov_valid:]] = -1
    m1_ovs[p_of[ov_valid:], w_of[ov_valid:]] = -1
    d_ovs[p_of[:n], w_of[:n]] = Dc[overflow]
    m0_ovs[p_of[:n], w_of[:n]] = m0[overflow]
    m1_ovs[p_of[:n], w_of[:n]] = m1[overflow]
    seg_ovs[p_of[:n], w_of[:n]] = seg[overflow]

    S_main = np.zeros((WMAIN, NPART, OC), dtype=np.float32)
    pp = np.arange(NPART)
    for wq in range(WMAIN):
        rg_, rep_ = wq // RCAP, wq % RCAP
        S_main[wq, pp, seg_main[:, rg_, rep_]] = d_main[:, rg_, rep_]
    S_ov = np.zeros((ovw, NPART, OC), dtype=np.float32)
    for wq in range(ovw):
        S_ov[wq, pp, seg_ovs[:, wq]] = d_ovs[:, wq]

    gw = GIDX // NPART
    m1_slot = m1_main.reshape(NPART, WMAIN)
    calls = np.zeros((WMAIN // gw, GIDX), dtype=np.int64)
    for call in range(WMAIN // gw):
        for g in range(gw):
            calls[call, g * NPART:(g + 1) * NPART] = m1_slot[:, call * gw + g]
    ov_calls0 = np.zeros((novc, GIDX), dtype=np.int64)
    ov_calls1 = np.zeros((novc, GIDX), dtype=np.int64)
    for k in range(novc):
        for g in range(gw):
            ov_calls0[k, g * NPART:(g + 1) * NPART] = m0_ovs[:, gw * k + g]
            ov_calls1[k, g * NPART:(g + 1) * NPART] = m1_ovs[:, gw * k + g]

    bf16 = ml_dtypes.bfloat16
    return {
        "wlin": np.ascontiguousarray(
            Wlin.reshape(RG, NPART, OC).transpose(1, 0, 2)).astype(bf16),
        "const": const.reshape(OC, 1),
        "s_main": np.ascontiguousarray(
            S_main.reshape(NCHUNK, WCH, NPART, OC).transpose(0, 2, 1, 3)
        ).astype(bf16),
        "s_ov": np.ascontiguousarray(
            S_ov.transpose(1, 0, 2)).astype(bf16),
        "idx1": np.ascontiguousarray(
            _wrap_idx_calls(calls)
            .reshape(NPART, NCHUNK, CPC * (GIDX // 16)).transpose(1, 0, 2)),
        "idx0ov": _wrap_idx_calls(ov_calls0),
        "idx1ov": _wrap_idx_calls(ov_calls1),
    }


def _overflow_slots(input_mask):
    worst = 0
    for core in range(NCORES):
        lo = core * TC
        m0 = input_mask[2 * lo:2 * (lo + TC):2].astype(np.int64)
        counts = np.bincount(m0, minlength=IN)
        worst = max(worst, int(np.maximum(counts - RCAP, 0).sum()))
    ov_valid = ((worst + 15) // 16) * 16
    ov_slots = max(GIDX, ((ov_valid + GIDX - 1) // GIDX) * GIDX)
    return ov_slots, ov_valid


def get_program(ov_slots, ov_valid):
    key = ("nc", ov_slots, ov_valid)
    if key not in _CACHE:
        _CACHE[key] = _build_program(ov_slots, ov_valid)
    return _CACHE[key]


def run(input, input_mask, lut_weights, bias, trace=False):
    from concourse.bass_utils import run_bass_kernel_spmd

    input = np.asarray(input)
    input_mask = np.asarray(input_mask)
    lut_weights = np.asarray(lut_weights)
    bias = np.asarray(bias)

    ov_slots, ov_valid = _overflow_slots(input_mask)
    nc = get_program(ov_slots, ov_valid)

    input_t = np.ascontiguousarray(input.T).astype(np.float32, copy=False)
    input_sb = np.ascontiguousarray(
        input_t.reshape(RG, NPART, B).transpose(1, 0, 2))
    in_maps = []
    for core in range(NCORES):
        m = _prep_core(core, input_mask, lut_weights, bias, ov_slots, ov_valid)
        m["input_t"] = input_t
        m["input_sb"] = input_sb
        m["input_bf"] = input_sb.astype(ml_dtypes.bfloat16)
        in_maps.append(m)

    res = run_bass_kernel_spmd(nc, in_maps, list(range(NCORES)), trace=trace)
    out = np.concatenate([r["out_c"].T for r in res.results], axis=1)
    return out.astype(np.float32, copy=False), res


def kernel(input, input_mask, lut_weights, bias):
    out, _ = run(input, input_mask, lut_weights, bias)
    return out


# revision 23
# speedup vs baseline: 1.0243x; 1.0243x over previous
"""Trainium2 Bass kernel for the LUT-linear (embedding_lookup) problem — V4.

Math: per_table[b,t] = (A + B*x0 + C*x1 + D*x0*x1)/4 with
x0 = input[b, mask[2t]], x1 = input[b, mask[2t+1]];
A=w0+w1+w2+w3, B=-w0+w1-w2+w3, C=-w0-w1+w2+w3, D=w0-w1-w2+w3.
out[b,o] = bias[o] + sum_{t in seg_o} per_table (segments = 512 contiguous
tables per out-feature).

Strategy (8 cores, table-sharded; input replicated):
  - Linear + const terms fold on the host into weight-only arrays:
    W_lin[i,o] = sum_t (B/4)[m0=i] + (C/4)[m1=i], const[o] = bias + sum A/4;
    applied with 4 PE matmuls. Only the quadratic term runs per-table.
  - Tables are placed into m0-runs: run i (= rg*128 + rr) owns RCAP=64
    slots at partition rr, w = rg*64+rep. x0 for every slot of run i is
    input[:, i] -> a stride-0 broadcast VIEW of input_sb[rr, rg, :]: no
    gather for x0. Run overflow (~1.7K tables/core) goes to an OV region
    where both x0 and x1 are SWDGE-gathered.
  - x1 is SWDGE-gathered per slot (32 main + 2 OV calls of 1024 idxs).
    Q7 descriptor-gen ucode is the critical path.
  - DVE: one pass per chunk, u = x0_view * x1 (f32 in, bf16 out).
  - Reduce: per-w stationary S_w[p,o] = D/4 routed to the slot's segment.
    Matmuls alternate PE column halves (even w -> PSUM rows 0:64, odd w
    -> rows 64:128) so each LDWEIGHTS overlaps the other half's MATMUL.
  - Epilogue: out = ps_even + ps_odd + const in one DVE op.
"""

import numpy as np
import ml_dtypes

NCORES = 8
B = 64
IN = 512
OUT = 512
T = IN * OUT
TC = T // NCORES          # 32768 tables per core
SEG = 512                 # tables per out-feature
OC = OUT // NCORES        # 64 out-features per core
NPART = 128
RG = IN // NPART          # 4 row-groups
RCAP = 64                 # slots per m0-run
WMAIN = RG * RCAP         # 256 main w-columns
NCHUNK = 8
WCH = WMAIN // NCHUNK     # 32 w-cols per chunk
GIDX = 1024               # idxs per dma_gather call
GW = GIDX // 128          # w-cols covered per gather call
CPC = (WMAIN // NCHUNK) // GW  # gather calls per chunk
NQUEUES = 4
OV_CHUNK_AFTER = 2        # run overflow compute after this chunk

_CACHE = {}


def _build_program(ov_slots, ov_valid):
    import concourse.bacc as bacc
    import concourse.mybir as mybir
    from concourse import library_config
    from concourse.tile import TileContext

    f32 = mybir.dt.float32
    bf16 = mybir.dt.bfloat16
    i16 = mybir.dt.int16
    Alu = mybir.AluOpType

    novc = ov_slots // GIDX
    ovw = ov_slots // NPART

    nc = bacc.Bacc("TRN2", target_bir_lowering=False, debug=False,
                   num_devices=NCORES, num_swdge_queues=NQUEUES,
                   dynamic_dma_scratch_size=65536)

    input_t = nc.dram_tensor("input_t", [IN, B], f32, kind="ExternalInput")
    input_sb_d = nc.dram_tensor("input_sb", [NPART, RG, B], f32, kind="ExternalInput")
    input_bf_d = nc.dram_tensor("input_bf", [NPART, RG, B], bf16, kind="ExternalInput")
    wlin_d = nc.dram_tensor("wlin", [NPART, RG, OC], bf16, kind="ExternalInput")
    const_d = nc.dram_tensor("const", [OC, 1], f32, kind="ExternalInput")
    s_main_d = nc.dram_tensor("s_main", [NCHUNK, NPART, WCH, OC], bf16, kind="ExternalInput")
    s_ov_d = nc.dram_tensor("s_ov", [NPART, ovw, OC], bf16, kind="ExternalInput")
    idx1_d = nc.dram_tensor("idx1", [NCHUNK, NPART, CPC * (GIDX // 16)], i16, kind="ExternalInput")
    idx0ov_d = nc.dram_tensor("idx0ov", [NPART, novc * (GIDX // 16)], i16, kind="ExternalInput")
    idx1ov_d = nc.dram_tensor("idx1ov", [NPART, novc * (GIDX // 16)], i16, kind="ExternalInput")
    out_d = nc.dram_tensor("out_c", [OC, B], f32, kind="ExternalOutput")

    # parity bookkeeping for the two PSUM column-halves
    first_even = [True]
    first_odd = [True]

    with TileContext(nc) as tc:
        nc.gpsimd.load_library(library_config.mlp)
        with (
            tc.tile_pool(name="idx", bufs=1) as idx_pool,
            tc.tile_pool(name="small", bufs=1) as small_pool,
            tc.tile_pool(name="x1", bufs=6) as x1_pool,
            tc.tile_pool(name="u", bufs=4) as u_pool,
            tc.tile_pool(name="s", bufs=3) as s_pool,
            tc.tile_pool(name="ov", bufs=1) as ov_pool,
            tc.tile_pool(name="psum", bufs=1, space="PSUM") as psum_pool,
        ):
            # overflow idx tiles load first so OV gathers can start early;
            # main idx is split per chunk so chunk 0 does not wait on all of it
            idx0ov_sb = idx_pool.tile([NPART, novc * (GIDX // 16)], i16, tag="idx0ov")
            idx1ov_sb = idx_pool.tile([NPART, novc * (GIDX // 16)], i16, tag="idx1ov")
            nc.sync.dma_start(idx0ov_sb[:], idx0ov_d[:])
            nc.sync.dma_start(idx1ov_sb[:], idx1ov_d[:])
            idx1_sbs = []
            for c in range(NCHUNK):
                t = idx_pool.tile([NPART, CPC * (GIDX // 16)], i16, tag=f"idx1_{c}")
                nc.sync.dma_start(t[:], idx1_d[c])
                idx1_sbs.append(t)

            input_sb = small_pool.tile([NPART, RG, B], f32, tag="input_sb")
            nc.sync.dma_start(input_sb[:], input_sb_d[:])
            wlin_sb = small_pool.tile([NPART, RG, OC], bf16, tag="wlin")
            nc.sync.dma_start(wlin_sb[:], wlin_d[:])
            const_sb = small_pool.tile([OC, 1], f32, tag="const")
            nc.sync.dma_start(const_sb[:], const_d[:])
            s_ov_sb = small_pool.tile([NPART, ovw, OC], bf16, tag="s_ov")
            nc.sync.dma_start(s_ov_sb[:], s_ov_d[:])

            ps = psum_pool.tile([NPART, B], f32, tag="ps")

            def quad_matmul(w_parity, s_ap, u_ap, last=False):
                if w_parity == 0:
                    o_ap = ps[0:OC, :]
                    start = first_even[0]
                    first_even[0] = False
                else:
                    o_ap = ps[OC:2 * OC, :]
                    start = first_odd[0]
                    first_odd[0] = False
                nc.tensor.matmul(o_ap, s_ap, u_ap, start=start, stop=last,
                                 skip_group_check=True)

            # warm-up: a 16-idx gather with a memset idx tile absorbs the
            # SWDGE ucode cold-start while the real idx tiles are still in DMA
            full_reg = nc.gpsimd.to_reg(GIDX)
            # valid counts per OV call (trailing -1 idxs are skipped by ucode)
            ov_counts = [min(GIDX, max(0, ov_valid - k * GIDX)) for k in range(novc)]

            # overflow tiles: memset first — slots beyond the valid count are
            # never written by the gather (-1 idxs) and must not feed NaNs
            # into u*0. The ov gathers themselves are emitted mid-stream
            # (after chunk 1) so chunk 0 compute starts as early as possible.
            x0ov = ov_pool.tile([NPART, ovw, B], f32, tag="x0ov")
            x1ov = ov_pool.tile([NPART, ovw, B], f32, tag="x1ov")
            nc.vector.memset(x0ov[:], 0.0)
            nc.vector.memset(x1ov[:], 0.0)
            qn = 0

            def emit_ov_gathers():
                nonlocal qn
                for k in range(novc):
                    cnt = ov_counts[k]
                    nc.gpsimd.dma_gather(
                        x0ov[:, k * GW:(k + 1) * GW, :], input_t[:],
                        idx0ov_sb[:, k * (GIDX // 16):(k + 1) * (GIDX // 16)],
                        GIDX, full_reg if cnt == GIDX else cnt, B,
                        queue_num=qn % NQUEUES)
                    qn += 1
                    nc.gpsimd.dma_gather(
                        x1ov[:, k * GW:(k + 1) * GW, :], input_t[:],
                        idx1ov_sb[:, k * (GIDX // 16):(k + 1) * (GIDX // 16)],
                        GIDX, full_reg if cnt == GIDX else cnt, B,
                        queue_num=qn % NQUEUES)
                    qn += 1

            emit_ov_gathers()

            # linear part: 4 matmuls on the even half (input_bf is host-cast)
            input_bf = small_pool.tile([NPART, RG, B], bf16, tag="input_bf")
            nc.sync.dma_start(input_bf[:], input_bf_d[:])
            for rg in range(RG):
                quad_matmul(0, wlin_sb[:, rg, :], input_bf[:, rg, :])

            for c in range(NCHUNK):
                s_sb = s_pool.tile([NPART, WCH, OC], bf16, tag="s")
                nc.sync.dma_start(s_sb[:], s_main_d[c])

                rg = c // (NCHUNK // RG)
                if c < NCHUNK - 1:
                    # bulk path: one DVE op per chunk
                    x1t = x1_pool.tile([NPART, WCH, B], f32, tag="x1")
                    for j in range(CPC):
                        nc.gpsimd.dma_gather(
                            x1t[:, j * GW:(j + 1) * GW, :], input_t[:],
                            idx1_sbs[c][:, j * (GIDX // 16):(j + 1) * (GIDX // 16)],
                            GIDX, full_reg, B, queue_num=qn % NQUEUES)
                        qn += 1
                    u = u_pool.tile([NPART, WCH, B], bf16, tag="u")
                    xv = input_sb[:, rg, :].unsqueeze(1).broadcast_to([NPART, WCH, B])
                    nc.vector.tensor_tensor(u[:], xv, x1t[:], Alu.mult)
                    for wl in range(WCH):
                        quad_matmul(wl % 2, s_sb[:, wl, :], u[:, wl, :])
                else:
                    # tail path: consume per gather call to shorten the tail
                    xvg = input_sb[:, rg, :].unsqueeze(1).broadcast_to([NPART, GW, B])
                    for j in range(CPC):
                        x1t = x1_pool.tile([NPART, GW, B], f32, tag="x1s")
                        nc.gpsimd.dma_gather(
                            x1t[:], input_t[:],
                            idx1_sbs[c][:, j * (GIDX // 16):(j + 1) * (GIDX // 16)],
                            GIDX, full_reg, B, queue_num=qn % NQUEUES)
                        qn += 1
                        u = u_pool.tile([NPART, GW, B], bf16, tag="us")
                        nc.vector.tensor_tensor(u[:], xvg, x1t[:], Alu.mult)
                        for wg in range(GW):
                            wl = j * GW + wg
                            last = (j == CPC - 1) and wg >= GW - 2
                            quad_matmul(wl % 2, s_sb[:, wl, :], u[:, wg, :], last=last)

                if c == OV_CHUNK_AFTER:
                    uov = ov_pool.tile([NPART, ovw, B], bf16, tag="uov")
                    nc.vector.tensor_tensor(uov[:], x0ov[:], x1ov[:], Alu.mult)
                    for wl in range(ovw):
                        quad_matmul(wl % 2, s_ov_sb[:, wl, :], uov[:, wl, :])

            out_sb = small_pool.tile([OC, B], f32, tag="out")
            nc.vector.tensor_scalar(out_sb[:], ps[0:OC, :], const_sb[:], None, Alu.add)
            nc.vector.tensor_tensor(out_sb[:], out_sb[:], ps[OC:2 * OC, :], Alu.add)
            nc.sync.dma_start(out_d[:], out_sb[:])

    nc.compile()
    return nc


def _wrap_idx_calls(vals):
    """vals [ncalls, 1024] (position order g*128+p) -> [128, ncalls*64] i16."""
    ncalls = vals.shape[0]
    w = vals.reshape(ncalls, GIDX // 16, 16).transpose(0, 2, 1)
    w = np.tile(w, (1, 8, 1))
    return np.ascontiguousarray(
        w.transpose(1, 0, 2).reshape(NPART, ncalls * (GIDX // 16))
    ).astype(np.int16)


def _prep_core(core, input_mask, lut_weights, bias, ov_slots, ov_valid):
    lo = core * TC
    m0 = input_mask[2 * lo:2 * (lo + TC):2].astype(np.int64)
    m1 = input_mask[2 * lo + 1:2 * (lo + TC):2].astype(np.int64)
    w = lut_weights[lo:lo + TC].astype(np.float32)
    A = (w[:, 0] + w[:, 1] + w[:, 2] + w[:, 3]) * 0.25
    Bc = (-w[:, 0] + w[:, 1] - w[:, 2] + w[:, 3]) * 0.25
    Cc = (-w[:, 0] - w[:, 1] + w[:, 2] + w[:, 3]) * 0.25
    Dc = (w[:, 0] - w[:, 1] - w[:, 2] + w[:, 3]) * 0.25
    seg = np.arange(TC) // SEG

    Wlin = np.zeros((IN, OC), dtype=np.float32)
    np.add.at(Wlin, (m0, seg), Bc)
    np.add.at(Wlin, (m1, seg), Cc)
    const = bias[core * OC:(core + 1) * OC].astype(np.float32).copy()
    np.add.at(const, seg, A)

    d_main = np.zeros((NPART, RG, RCAP), dtype=np.float32)
    m1_main = np.zeros((NPART, RG, RCAP), dtype=np.int64)
    seg_main = np.zeros((NPART, RG, RCAP), dtype=np.int64)
    order = np.argsort(m0, kind="stable")
    counts = np.bincount(m0, minlength=IN)
    starts = np.zeros(IN + 1, dtype=np.int64)
    np.cumsum(counts, out=starts[1:])
    overflow = []
    for i in range(IN):
        tabs = order[starts[i]:starts[i + 1]]
        rr, rg = i % NPART, i // NPART
        fill = min(len(tabs), RCAP)
        tk = tabs[:fill]
        d_main[rr, rg, :fill] = Dc[tk]
        m1_main[rr, rg, :fill] = m1[tk]
        seg_main[rr, rg, :fill] = seg[tk]
        overflow.extend(tabs[RCAP:])
    overflow = np.asarray(overflow, dtype=np.int64)

    assert len(overflow) <= ov_valid <= ov_slots, (len(overflow), ov_valid, ov_slots)
    novc = ov_slots // GIDX
    ovw = ov_slots // NPART
    n = len(overflow)
    # pad with duplicate-dummy valid entries (idx 0, d=0) up to ov_valid so
    # every core shares the same per-call valid count; -1 beyond
    f = np.arange(ov_slots)
    p_of = f % NPART
    w_of = GW * (f // GIDX) + (f % GIDX) // NPART
    d_ovs = np.zeros((NPART, ovw), dtype=np.float32)
    m0_ovs = np.zeros((NPART, ovw), dtype=np.int64)
    m1_ovs = np.zeros((NPART, ovw), dtype=np.int64)
    seg_ovs = np.zeros((NPART, ovw), dtype=np.int64)
    m0_ovs[p_of[ov_valid:], w_of[ov_valid:]] = -1
    m1_ovs[p_of[ov_valid:], w_of[ov_valid:]] = -1
    d_ovs[p_of[:n], w_of[:n]] = Dc[overflow]
    m0_ovs[p_of[:n], w_of[:n]] = m0[overflow]
    m1_ovs[p_of[:n], w_of[:n]] = m1[overflow]
    seg_ovs[p_of[:n], w_of[:n]] = seg[overflow]

    S_main = np.zeros((WMAIN, NPART, OC), dtype=np.float32)
    pp = np.arange(NPART)
    for wq in range(WMAIN):
        rg_, rep_ = wq // RCAP, wq % RCAP
        S_main[wq, pp, seg_main[:, rg_, rep_]] = d_main[:, rg_, rep_]
    S_ov = np.zeros((ovw, NPART, OC), dtype=np.float32)
    for wq in range(ovw):
        S_ov[wq, pp, seg_ovs[:, wq]] = d_ovs[:, wq]

    gw = GIDX // NPART
    m1_slot = m1_main.reshape(NPART, WMAIN)
    calls = np.zeros((WMAIN // gw, GIDX), dtype=np.int64)
    for call in range(WMAIN // gw):
        for g in range(gw):
            calls[call, g * NPART:(g + 1) * NPART] = m1_slot[:, call * gw + g]
    ov_calls0 = np.zeros((novc, GIDX), dtype=np.int64)
    ov_calls1 = np.zeros((novc, GIDX), dtype=np.int64)
    for k in range(novc):
        for g in range(gw):
            ov_calls0[k, g * NPART:(g + 1) * NPART] = m0_ovs[:, gw * k + g]
            ov_calls1[k, g * NPART:(g + 1) * NPART] = m1_ovs[:, gw * k + g]

    bf16 = ml_dtypes.bfloat16
    return {
        "wlin": np.ascontiguousarray(
            Wlin.reshape(RG, NPART, OC).transpose(1, 0, 2)).astype(bf16),
        "const": const.reshape(OC, 1),
        "s_main": np.ascontiguousarray(
            S_main.reshape(NCHUNK, WCH, NPART, OC).transpose(0, 2, 1, 3)
        ).astype(bf16),
        "s_ov": np.ascontiguousarray(
            S_ov.transpose(1, 0, 2)).astype(bf16),
        "idx1": np.ascontiguousarray(
            _wrap_idx_calls(calls)
            .reshape(NPART, NCHUNK, CPC * (GIDX // 16)).transpose(1, 0, 2)),
        "idx0ov": _wrap_idx_calls(ov_calls0),
        "idx1ov": _wrap_idx_calls(ov_calls1),
    }


def _overflow_slots(input_mask):
    worst = 0
    for core in range(NCORES):
        lo = core * TC
        m0 = input_mask[2 * lo:2 * (lo + TC):2].astype(np.int64)
        counts = np.bincount(m0, minlength=IN)
        worst = max(worst, int(np.maximum(counts - RCAP, 0).sum()))
    ov_valid = ((worst + 15) // 16) * 16
    ov_slots = max(GIDX, ((ov_valid + GIDX - 1) // GIDX) * GIDX)
    return ov_slots, ov_valid


def get_program(ov_slots, ov_valid):
    key = ("nc", ov_slots, ov_valid)
    if key not in _CACHE:
        _CACHE[key] = _build_program(ov_slots, ov_valid)
    return _CACHE[key]


def run(input, input_mask, lut_weights, bias, trace=False):
    from concourse.bass_utils import run_bass_kernel_spmd

    input = np.asarray(input)
    input_mask = np.asarray(input_mask)
    lut_weights = np.asarray(lut_weights)
    bias = np.asarray(bias)

    ov_slots, ov_valid = _overflow_slots(input_mask)
    nc = get_program(ov_slots, ov_valid)

    input_t = np.ascontiguousarray(input.T).astype(np.float32, copy=False)
    input_sb = np.ascontiguousarray(
        input_t.reshape(RG, NPART, B).transpose(1, 0, 2))
    in_maps = []
    for core in range(NCORES):
        m = _prep_core(core, input_mask, lut_weights, bias, ov_slots, ov_valid)
        m["input_t"] = input_t
        m["input_sb"] = input_sb
        m["input_bf"] = input_sb.astype(ml_dtypes.bfloat16)
        in_maps.append(m)

    res = run_bass_kernel_spmd(nc, in_maps, list(range(NCORES)), trace=trace)
    out = np.concatenate([r["out_c"].T for r in res.results], axis=1)
    return out.astype(np.float32, copy=False), res


def kernel(input, input_mask, lut_weights, bias):
    out, _ = run(input, input_mask, lut_weights, bias)
    return out
